# revision 2
# baseline (speedup 1.0000x reference)
"""BaiLing attention Trainium2 kernel (v2).

Sharding: 8 cores = 2 (batch) x 4 (tensor-parallel over heads).
Each TP rank r owns q heads 4r..4r+3 and kv head r (GQA group-aligned),
computes its out-projection partial; host sums the 4 partials per batch.

v2 changes vs the 407us baseline (all PE-row / ACT-table driven):
  - attention data path in bf16 (q, k, v^T, ep, oT, wo): short-N
    matmuls run 1 cycle/row at any N (f32r needs N>=256 and even N).
  - softmax denominator comes FREE from the PV matmul: ep tiles are the
    STATIONARY operand, v^T carries an appended ones-column, so psum
    col 128 accumulates sum(exp). This deletes the per-k-tile
    denominator matmuls (~82k rows) and the 1/den broadcast matmuls.
  - PV output lands [sq, d+1]: normalize is a per-partition DVE
    reciprocal + tensor_scalar multiply (no ACT Ln/Exp chain, so the
    attention phase is Exp-only on ACT: zero activation-table reloads).
    A bf16 PE transpose restores [d, sq] for the out-projection.
  - causal diagonal is trapezoid-sliced: scores/exp/PV only touch
    sq >= sk tiles; the triangular 128x128 mask rides a bf16
    identity-matmul on just the diagonal subtile.
  - QKV phase batches the per-chunk RMS Ln/Exp per seq-tile (2
    activation-table loads per tile instead of 10).
  - out partials stored bf16 (halves output DMA); host sums in f64.
"""

import sys

sys.path.insert(0, "/opt/trn_rl_repo")

import math
from contextlib import ExitStack

import numpy as np
import ml_dtypes

import concourse.bass as bass
import concourse.mybir as mybir
import concourse.tile as tile
from concourse import bacc
from concourse.bass_utils import run_bass_kernel_spmd

F32 = mybir.dt.float32
F32R = mybir.dt.float32r
BF16 = mybir.dt.bfloat16
I32 = mybir.dt.int32
AF = mybir.ActivationFunctionType
OP = mybir.AluOpType

H = 2048          # hidden size
S = 2048          # sequence length
D = 128           # head dim
NH_L = 4          # q heads per rank
QW = NH_L * D     # 512 local q width
CW = QW + 2 * D   # 768 local qkv width
P = 128
KO = H // P       # 16 contraction tiles
ST = S // 512     # 4 seq tiles of 512
SM_SCALE = float(D) ** -0.5
EPS = 1e-6
ROPE_THETA = 10000.0
NEG = -1.0e9

# Cody-Waite split of 2*pi (short-mantissa terms: k*ci exact for k<=512)
CW1 = 6.28125
CW2 = 0.0019350051879882812
CW3 = 3.019916050561733e-07
PI = math.pi


def _build():
    nc = bacc.Bacc("TRN2", target_bir_lowering=False, debug=False, num_devices=8)

    hT = nc.dram_tensor("hT", [H, S], BF16, kind="ExternalInput").ap()
    wqkv = nc.dram_tensor("wqkv", [H, CW], BF16, kind="ExternalInput").ap()
    wo = nc.dram_tensor("wo", [QW, H], BF16, kind="ExternalInput").ap()
    pos = nc.dram_tensor("pos", [1, S], I32, kind="ExternalInput").ap()
    invf2 = nc.dram_tensor("invf2", [P, 1], F32, kind="ExternalInput").ap()
    wqn = nc.dram_tensor("wqn", [D, 1], F32, kind="ExternalInput").ap()
    wkn = nc.dram_tensor("wkn", [D, 1], F32, kind="ExternalInput").ap()
    masktri = nc.dram_tensor("masktri", [P, P], BF16, kind="ExternalInput").ap()
    ones = nc.dram_tensor("ones", [P, 1], F32, kind="ExternalInput").ap()
    ones16 = nc.dram_tensor("ones16", [P, KO], BF16, kind="ExternalInput").ap()
    onesrow16 = nc.dram_tensor("onesrow16", [1, P], BF16,
                               kind="ExternalInput").ap()
    ident16 = nc.dram_tensor("ident16", [P, P], BF16, kind="ExternalInput").ap()
    rmat = nc.dram_tensor("rmat", [P, P], F32, kind="ExternalInput").ap()
    out = nc.dram_tensor("out", [S, H], BF16, kind="ExternalOutput").ap()

    hT3 = hT.rearrange("(ko p) s -> p ko s", p=P)
    wqkv3 = wqkv.rearrange("(ko p) c -> p ko c", p=P)
    wo3 = wo.rearrange("(ks p) n -> p ks n", p=P)
    out3 = out.rearrange("(t p) n -> p t n", p=P)

    with ExitStack() as ctx:
        tc = ctx.enter_context(tile.TileContext(nc))
        consts = ctx.enter_context(tc.tile_pool(name="consts", bufs=1))
        qkvp = ctx.enter_context(tc.tile_pool(name="qkvp", bufs=1))
        vtp = ctx.enter_context(tc.tile_pool(name="vtp", bufs=1))

        abp = ExitStack()
        csp = abp.enter_context(tc.tile_pool(name="csp", bufs=1))

        # constants on the gpsimd (SWDGE) queue; sync queue starts on weights
        ones_sb = consts.tile([P, 1], F32R)
        nc.gpsimd.dma_start(ones_sb, ones.bitcast(F32R))
        onesrow_sb = consts.tile([1, P], BF16)
        nc.gpsimd.dma_start(onesrow_sb, onesrow16)
        masktri_sb = consts.tile([P, P], BF16)
        nc.gpsimd.dma_start(masktri_sb, masktri)
        ident16_sb = consts.tile([P, P], BF16)
        nc.gpsimd.dma_start(ident16_sb, ident16)
        rmat_sb = consts.tile([P, P], F32R)
        nc.gpsimd.dma_start(rmat_sb, rmat.bitcast(F32R))
        eps_sb = consts.tile([1, 1], F32)
        nc.vector.memset(eps_sb, EPS)
        wqn_sb = consts.tile([D, 1], F32)
        nc.gpsimd.dma_start(wqn_sb, wqn)
        wkn_sb = consts.tile([D, 1], F32)
        nc.gpsimd.dma_start(wkn_sb, wkn)

        q_sb = qkvp.tile([P, NH_L, S], BF16)
        k_sb = qkvp.tile([P, S], BF16)
        vT_sb = vtp.tile([P, KO, P + 1], BF16)
        # ones column for the free softmax denominator
        nc.gpsimd.dma_start(
            vT_sb[:, :, P : P + 1],
            ones16.rearrange("p (k o) -> p k o", o=1),
        )
        cosb = csp.tile([P, S], F32)
        sinb = csp.tile([P, S], F32)

        wqkv_p = abp.enter_context(tc.tile_pool(name="wqkv_p", bufs=1))
        ht_p = abp.enter_context(tc.tile_pool(name="ht_p", bufs=4))
        ropep = abp.enter_context(tc.tile_pool(name="ropep", bufs=1))
        xsp = abp.enter_context(tc.tile_pool(name="xsp", bufs=10))
        rowp = abp.enter_context(tc.tile_pool(name="rowp", bufs=6))
        cpool = abp.enter_context(tc.tile_pool(name="cpool", bufs=1))

        # ------------- Phase B: rope tables (cos/sin) --------------
        # computed packed on [128, 1024] tiles (full lane width): row
        # f + 64*b holds freq f, seq half b
        HS = S // 2
        with nc.named_scope("rope_tables"):
            pos_pk = ropep.tile([P, HS], I32, tag="tmp_a", name="pos_pk")
            for b in (0, 1):
                nc.gpsimd.dma_start(
                    pos_pk[64 * b : 64 * b + 64],
                    bass.AP(tensor=pos.tensor, offset=pos.offset + HS * b,
                            ap=[[0, 64], [1, HS]]),
                )
            ang = ropep.tile([P, HS], F32, tag="ang", name="ang")
            nc.vector.tensor_copy(ang, pos_pk)
            invf_sb = ropep.tile([P, 1], F32, tag="invf", name="invf_sb")
            nc.gpsimd.dma_start(invf_sb, invf2)
            nc.vector.tensor_scalar_mul(ang, ang, invf_sb)
            kt = ropep.tile([P, HS], F32, tag="kt", name="kt")
            nc.vector.tensor_scalar_mul(kt, ang, 1.0 / (2 * PI))
            ki = ropep.tile([P, HS], I32, tag="ki", name="ki")
            nc.vector.tensor_copy(ki, kt)
            nc.vector.tensor_copy(kt, ki)
            for c in (CW1, CW2, CW3):
                nc.vector.scalar_tensor_tensor(
                    out=ang, in0=kt, scalar=-c, in1=ang, op0=OP.mult, op1=OP.add
                )
            mgt = ropep.tile([P, HS], F32, tag="tmp_a", name="mgt")
            nc.vector.tensor_scalar(
                out=mgt, in0=ang, scalar1=PI, scalar2=None, op0=OP.is_gt
            )
            nc.vector.scalar_tensor_tensor(
                out=ang, in0=mgt, scalar=-2 * PI, in1=ang, op0=OP.mult, op1=OP.add
            )
            nc.vector.tensor_scalar(
                out=mgt, in0=ang, scalar1=-PI, scalar2=None, op0=OP.is_lt
            )
            nc.vector.scalar_tensor_tensor(
                out=ang, in0=mgt, scalar=2 * PI, in1=ang, op0=OP.mult, op1=OP.add
            )
            rc = ropep.tile([P, HS], F32, tag="ki", name="rc")
            nc.vector.tensor_scalar(
                out=mgt, in0=ang, scalar1=PI / 2, scalar2=PI,
                op0=OP.add, op1=OP.is_gt,
            )
            nc.vector.tensor_scalar_add(rc, ang, PI / 2)
            nc.vector.scalar_tensor_tensor(
                out=rc, in0=mgt, scalar=-2 * PI, in1=rc, op0=OP.mult, op1=OP.add
            )
            # unpack into the [128, S] tables (both d-halves identical)
            for half in (0, 64):
                for b in (0, 1):
                    src = slice(64 * b, 64 * b + 64)
                    dst_r = slice(half, half + 64)
                    dst_c = slice(HS * b, HS * b + HS)
                    nc.scalar.activation(sinb[dst_r, dst_c], ang[src], AF.Sin)
                    nc.scalar.activation(cosb[dst_r, dst_c], rc[src], AF.Sin)

        # ---- Phase A: QKV projection + fused norm/rope/v-transpose ----
        CT_ORDER = [4, 5, 0, 1, 2, 3]  # k, v, then q heads
        POOLS = {}

        with nc.named_scope("qkv_proj"):
            with tc.tile_pool(name="ps_a", bufs=4, space="PSUM") as ps_a, \
                 tc.tile_pool(name="ps_c", bufs=2, space="PSUM") as ps_c, \
                 tc.tile_pool(name="ps_s", bufs=2, space="PSUM") as ps_s:
                POOLS["late"] = (ps_c, "qr")
                wq_sb = wqkv_p.tile([P, KO, CW], BF16)
                ones16b = consts.tile([P, 1], BF16)
                nc.gpsimd.dma_start(ones16b, ones16[:, 0:1])

                def fin_b(x, ri, w_sb, dst, ssl):
                    # x*w (per-partition scale on ACT), rope, rms-normalize
                    pool, tag = POOLS["late"]
                    nc.scalar.activation(x, x, AF.Copy, scale=w_sb)
                    t1m = cpool.tile([P, 512], F32, tag="t1m", bufs=2,
                                     name="t1m")
                    nc.vector.tensor_mul(t1m, x, cosb[:, ssl])
                    qr = pool.tile([P, 512], F32, tag=tag, name="qr")
                    nc.tensor.matmul(qr, rmat_sb, x, start=True, stop=True)
                    nc.vector.tensor_tensor(x, qr, sinb[:, ssl], OP.mult)
                    nc.vector.tensor_tensor(x, x, t1m, OP.add)
                    rb = pool.tile([P, 512], F32, tag=tag, name="rb")
                    nc.tensor.matmul(rb, onesrow_sb, ri, start=True, stop=True)
                    nc.vector.tensor_tensor(dst, x, rb, OP.mult)

                fins = []
                for st in range(ST):
                    ssl = slice(512 * st, 512 * (st + 1))
                    hts = []
                    for g in range(4):  # quarter-K chunks of 4 ko each
                        ht_sb = ht_p.tile([P, KO // 4, 512], BF16, tag="ht",
                                          name=f"ht_{st}_{g}")
                        if st == 0:
                            # k/v weight columns (1MB bf16) stream first so
                            # the first psum groups aren't gated on the full
                            # weight load; q columns follow
                            ks = slice(4 * g, 4 * g + 4)
                            nc.sync.dma_start(
                                wq_sb[:, ks, QW:CW], wqkv3[:, ks, QW:CW])
                        nc.sync.dma_start(
                            ht_sb, hT3[:, 4 * g : 4 * g + 4, ssl])
                        hts.append(ht_sb)
                    if st == 0:
                        for g in range(4):
                            ks = slice(4 * g, 4 * g + 4)
                            nc.sync.dma_start(
                                wq_sb[:, ks, 0:QW], wqkv3[:, ks, 0:QW])
                    ln5 = rowp.tile([1, 5, 512], BF16, tag="ln", bufs=2,
                                    name="ln5")
                    pend_a = []
                    for ct in CT_ORDER:
                        acc = ps_a.tile([P, 512], F32, tag="qkv_ps",
                                        name=f"qkv_ps_{st}_{ct}")
                        for ko in range(KO):
                            nc.tensor.matmul(
                                acc,
                                wq_sb[:, ko, P * ct : P * (ct + 1)],
                                hts[ko // 4][:, ko % 4],
                                start=(ko == 0),
                                stop=(ko == KO - 1),
                            )
                        if ct == 5:
                            vch = cpool.tile([P, 512], BF16, tag="vch", bufs=2,
                                             name="vch")
                            nc.vector.tensor_copy(vch, acc)
                            for i in range(4):
                                vt_ps = ps_c.tile([P, P], BF16, tag="qr",
                                                  name="vt_ps")
                                nc.tensor.transpose(
                                    vt_ps, vch[:, P * i : P * (i + 1)],
                                    ident16_sb)
                                nc.vector.tensor_copy(
                                    vT_sb[:, 4 * st + i, 0:P], vt_ps)
                        else:
                            x = xsp.tile([P, 512], F32R, tag="x",
                                         name=f"x_{st}_{ct}")
                            nc.vector.tensor_copy(x, acc)
                            xsq = cpool.tile([P, 512], BF16, tag="xsq", bufs=2,
                                             name="xsq")
                            nc.vector.tensor_mul(xsq, x, x)
                            ssq = ps_s.tile([1, 512], F32, tag="s", name="ssq")
                            nc.tensor.matmul(ssq, ones16b, xsq,
                                             start=True, stop=True)
                            ci = len(pend_a)
                            nc.scalar.activation(ln5[:, ci], ssq, AF.Ln,
                                                 bias=eps_sb, scale=1.0 / D)
                            if ct == 4:
                                pend_a.append((x, wkn_sb, k_sb[:, ssl]))
                            else:
                                pend_a.append((x, wqn_sb, q_sb[:, ct, ssl]))
                        if fins:
                            fins.pop(0)()
                    # batched Exp for the 5 rms rows as ONE instruction
                    ri5 = rowp.tile([1, 5, 512], BF16, tag="ri", bufs=2,
                                    name="ri5")
                    nc.scalar.activation(ri5, ln5, AF.Exp, scale=-0.5)
                    newf = []
                    for ci, (x, w_sb, dst) in enumerate(pend_a):
                        newf.append(
                            lambda x=x, ri=ri5[:, ci], w_sb=w_sb, dst=dst,
                            ssl=ssl: fin_b(x, ri, w_sb, dst, ssl))
                    while fins:
                        fins.pop(0)()
                    fins = newf
                # st3's fins are drained inside the first attention heads

        # ------------- Phase E/F: attention + out projection -------------
        with tc.tile_pool(name="otp", bufs=1) as otp, \
             tc.tile_pool(name="wop", bufs=1) as wop, \
             tc.tile_pool(name="expp", bufs=6) as expp, \
             tc.tile_pool(name="finp", bufs=8) as finp, \
             tc.tile_pool(name="outp", bufs=3) as outp, \
             tc.tile_pool(name="ps_sc", bufs=3, space="PSUM") as ps_sc, \
             tc.tile_pool(name="ps_o", bufs=4, space="PSUM") as ps_o:
            POOLS["late"] = (ps_sc, "sc")
            oT_sb = otp.tile([P, NH_L, S], BF16)
            wo_sb = wop.tile([P, NH_L, H], BF16)
            for ks in range(NH_L):
                nc.gpsimd.dma_start(wo_sb[:, ks], wo3[:, ks])
            pend_hooks = list(fins)

            def attn_head(st, hh):
                """Scores/exp/PV for one (seq-tile, head); returns a
                closure draining the deferred transposes. One pend_hooks
                entry drains per k-tile, interleaving deferred work into
                the scores/PV stream. Each o-accumulator owns a FULL psum
                bank: a bank supports only one open accumulation group, so
                two interleaved groups in one bank corrupt the first."""
                n_sk = 4 * st + 4
                qh = q_sb[:, hh, 512 * st : 512 * (st + 1)]
                ocs = [ps_o.tile([P, P + 4], F32, tag="o",
                                 padded_shape=[P, 512],
                                 name=f"o_{st}_{hh}_{c}") for c in range(4)]
                pend_fb = []

                def fin_a(c):
                    # group c just closed: normalize out of psum (frees the
                    # bank), defer the transpose one step
                    t = ocs[c]
                    rinv = finp.tile([P, 1], F32, tag="rinv", name="rinv")
                    nc.vector.reciprocal(rinv, t[:, P : P + 1])
                    o_c = finp.tile([P, P], BF16, tag="oc", name="o_c")
                    nc.vector.tensor_scalar_mul(o_c, t[:, 0:P], rinv)

                    def fin_b(c=c, o_c=o_c):
                        tp = ps_sc.tile([P, P], BF16, tag="sc", name="tp")
                        nc.tensor.transpose(tp, o_c, ident16_sb)
                        nc.scalar.copy(
                            oT_sb[:, hh,
                                  512 * st + P * c : 512 * st + P * (c + 1)],
                            tp)
                    pend_fb.append(fin_b)

                def emit_pv(j, ep):
                    if pend_fb:
                        pend_fb.pop(0)()
                    for c in range(max(0, j - 4 * st), 4):
                        nc.tensor.matmul(
                            ocs[c][:, 0 : P + 1], ep[:, P * c : P * (c + 1)],
                            vT_sb[:, j],
                            start=(j == 0), stop=(j == 4 * st + c),
                        )
                    if j >= 4 * st:
                        fin_a(j - 4 * st)

                pend = []
                for j in range(n_sk):
                    sT = ps_sc.tile([P, 512], F32, tag="sc", name="sT")
                    if j < 4 * st:
                        lo = 0
                        nc.tensor.matmul(sT, k_sb[:, P * j : P * (j + 1)], qh,
                                         start=True, stop=True)
                    else:
                        i = j - 4 * st
                        lo = P * i
                        sub = slice(P * i, P * (i + 1))
                        nc.tensor.matmul(sT[:, sub], ident16_sb, masktri_sb,
                                         start=True, stop=False)
                        nc.tensor.matmul(sT[:, sub],
                                         k_sb[:, P * j : P * (j + 1)],
                                         qh[:, sub], start=False, stop=True)
                        if i < 3:
                            rest = slice(P * (i + 1), 512)
                            nc.tensor.matmul(sT[:, rest],
                                             k_sb[:, P * j : P * (j + 1)],
                                             qh[:, rest],
                                             start=True, stop=True)
                    ep = expp.tile([P, 512], BF16, tag="ep", name="ep")
                    nc.scalar.activation(ep[:, lo:512], sT[:, lo:512], AF.Exp,
                                         scale=SM_SCALE)
                    if j >= 1 and pend_hooks:
                        pend_hooks.pop(0)()
                    pend.append((j, ep))
                    if len(pend) > 2:
                        emit_pv(*pend.pop(0))
                for pe_args in pend:
                    emit_pv(*pe_args)

                def finalize():
                    while pend_fb:
                        pend_fb.pop(0)()
                return finalize

            def out_proj(st):
                tail = st == ST - 1
                with nc.named_scope(f"out_proj_t{st}"):
                    for t in range(4 * st, 4 * st + 4):
                        out_sb = outp.tile([P, H], BF16, tag="out_sb",
                                           name="out_sb")
                        for nt in range(4):
                            acc = ps_sc.tile([P, 512], F32, tag="sc",
                                             name="out_ps")
                            for ks in range(NH_L):
                                nc.tensor.matmul(
                                    acc,
                                    oT_sb[:, ks, P * t : P * (t + 1)],
                                    wo_sb[:, ks, 512 * nt : 512 * (nt + 1)],
                                    start=(ks == 0),
                                    stop=(ks == NH_L - 1),
                                )
                            osl = out_sb[:, 512 * nt : 512 * (nt + 1)]
                            if nt % 2 == 1:
                                nc.scalar.copy(osl, acc)
                            else:
                                nc.vector.tensor_copy(osl, acc)
                            if tail:
                                nc.sync.dma_start(
                                    out3[:, t, 512 * nt : 512 * (nt + 1)], osl)
                        if not tail:
                            nc.sync.dma_start(out3[:, t], out_sb)

            prev_F = None
            for st in range(ST):
                fins_st = []
                for hh in range(NH_L):
                    if hh == 0:
                        if prev_F is not None:
                            pend_hooks.append(prev_F)
                            prev_F = None
                    else:
                        pend_hooks.append(fins_st[hh - 1])
                    with nc.named_scope(f"attn_h{hh}_t{st}"):
                        fins_st.append(attn_head(st, hh))
                if st == 0:
                    while pend_hooks:
                        pend_hooks.pop(0)()

                def flush(st=st, lf=fins_st[3]):
                    lf()
                    out_proj(st)
                prev_F = flush
            while pend_hooks:
                pend_hooks.pop(0)()
            prev_F()

        abp.close()

    nc.compile()
    return nc


_NC_CACHE = None


def _get_nc():
    global _NC_CACHE
    if _NC_CACHE is None:
        _NC_CACHE = _build()
    return _NC_CACHE


def _host_inputs(positions, hidden_states, w_qkv, w_o, q_norm_w, k_norm_w):
    """Build the 8 per-core input maps."""
    positions = np.asarray(positions, dtype=np.int32)
    hidden_states = np.asarray(hidden_states, dtype=np.float32)
    w_qkv = np.asarray(w_qkv, dtype=np.float32)
    w_o = np.asarray(w_o, dtype=np.float32)
    q_norm_w = np.asarray(q_norm_w, dtype=np.float32)
    k_norm_w = np.asarray(k_norm_w, dtype=np.float32)

    invf = (
        1.0 / (ROPE_THETA ** (np.arange(0, D, 2, dtype=np.float32) / D))
    ).astype(np.float32).reshape(64, 1)
    invf2 = np.concatenate([invf, invf], axis=0)
    p_idx = np.arange(P).reshape(P, 1)
    c_idx = np.arange(P).reshape(1, P)
    masktri = np.where(p_idx > c_idx, np.float32(NEG),
                       np.float32(0.0)).astype(ml_dtypes.bfloat16)
    ones = np.ones((P, 1), dtype=np.float32)
    ones16 = np.ones((P, KO), dtype=np.float32).astype(ml_dtypes.bfloat16)
    onesrow16 = np.ones((1, P), dtype=np.float32).astype(ml_dtypes.bfloat16)
    ident16 = np.eye(P, dtype=np.float32).astype(ml_dtypes.bfloat16)
    rmat = np.zeros((P, P), dtype=np.float32)
    for i in range(64):
        rmat[64 + i, i] = -1.0
        rmat[i, 64 + i] = 1.0
    wqn = q_norm_w.reshape(D, 1)
    wkn = k_norm_w.reshape(D, 1)

    in_maps = []
    for core in range(8):
        g, r = core // 4, core % 4
        wq_cols = w_qkv[:, 512 * r : 512 * (r + 1)]
        wk_col = w_qkv[:, 2048 + 128 * r : 2048 + 128 * (r + 1)]
        wv_col = w_qkv[:, 2560 + 128 * r : 2560 + 128 * (r + 1)]
        in_maps.append(
            {
                "hT": np.ascontiguousarray(hidden_states[g].T).astype(
                    ml_dtypes.bfloat16),
                "wqkv": np.ascontiguousarray(
                    np.concatenate([wq_cols, wk_col, wv_col], axis=1)
                ).astype(ml_dtypes.bfloat16),
                "wo": np.ascontiguousarray(
                    w_o[512 * r : 512 * (r + 1), :]
                ).astype(ml_dtypes.bfloat16),
                "pos": positions[g : g + 1],
                "invf2": invf2,
                "wqn": wqn,
                "wkn": wkn,
                "masktri": masktri,
                "ones": ones,
                "ones16": ones16,
                "onesrow16": onesrow16,
                "ident16": ident16,
                "rmat": rmat,
            }
        )
    return in_maps


def run(trace=False, **inputs):
    nc = _get_nc()
    in_maps = _host_inputs(**inputs)
    res = run_bass_kernel_spmd(nc, in_maps, core_ids=list(range(8)),
                               trace=trace)
    B = inputs["hidden_states"].shape[0]
    out = np.zeros((B, S, H), dtype=np.float64)
    for core in range(8):
        g = core // 4
        out[g] += res.results[core]["out"].astype(np.float64)
    return out.astype(np.float32), res


def kernel(**inputs):
    out, _ = run(trace=False, **inputs)
    return out
